# revision 7
# baseline (speedup 1.0000x reference)
"""Multi-head attention Trainium2 kernel (B=2, S=2048, D=1024, H=16).

Sharding: 8 cores, each handles (batch b = core//4, head group g = core%4,
heads 4g..4g+3). Returns full (out, attn) like the reference.

Per-core plan (matmuls in fp32r = 1 cyc/row at N>=256):
  phase 0: masks/biases, identity, weight transposes (PE)
  phase 1: transpose x_b -> xT [1024, 2048] via PE
  phase 2: projections q'T/k'T (per-head padded [128,2048] tiles, ones/mask
           row at partition 64), v natural [2048, 256]
  phase A (per head): natural logits = q'T.T @ k'T (K=128 incl. mask row),
           exp on ACT with accum_out row sums, normalize on DVE, DMA attn out
  phase B (per head): transposed logits kT.T@qT per k-chunk, exp with
           per-partition mask bias, AV matmuls accumulate ctxT in PSUM,
           scale by broadcast recip rows
  phase C: out_partial = ctxT.T @ denseT, DMA out
Host: sums the 4 partial outs per batch (+ constant bias term), stacks attn.
"""
import os
import sys

if "/opt/trn_rl_repo" not in sys.path:
    sys.path.insert(0, "/opt/trn_rl_repo")

import numpy as np

import concourse.bass as bass
import concourse.tile as tile
from concourse import mybir, bacc
from concourse.bass_utils import run_bass_kernel_spmd
from concourse.masks import make_identity

F32 = mybir.dt.float32
F32R = mybir.dt.float32r
I32 = mybir.dt.int32

B, S, D, H = 2, 2048, 1024, 16
DH = 64            # head dim
HPC = 4            # heads per core
JC = HPC * DH      # 256 j-dims per core
NCORES = 8
NEG = -1.0e9       # additive mask constant (pre-scale)
SCALE = 0.125      # 1/sqrt(64)

_NC_CACHE = None


def build_nc():
    nc = bacc.Bacc()

    x = nc.dram_tensor("x", [S, D], F32, kind="ExternalInput")
    maskb = nc.dram_tensor("maskb", [S], I32, kind="ExternalInput")
    wq = nc.dram_tensor("wq", [JC, D], F32, kind="ExternalInput")
    wk = nc.dram_tensor("wk", [JC, D], F32, kind="ExternalInput")
    wv = nc.dram_tensor("wv", [JC, D], F32, kind="ExternalInput")
    wqb = nc.dram_tensor("wqb", [JC], F32, kind="ExternalInput")
    wkb = nc.dram_tensor("wkb", [JC], F32, kind="ExternalInput")
    dns = nc.dram_tensor("dns", [D, JC], F32, kind="ExternalInput")
    attn_p = nc.dram_tensor("attn_p", [HPC, S, S], F32, kind="ExternalOutput")
    out_p = nc.dram_tensor("out_p", [S, D], F32, kind="ExternalOutput")

    ST = S // 128   # 16 s-tiles
    IC = D // 128   # 8 input-feature chunks

    with tile.TileContext(nc) as tc:
        with (
            tc.tile_pool(name="life", bufs=1) as pp,
            tc.tile_pool(name="dram", bufs=1, space="DRAM") as dpool,
        ):
            ident = pp.tile([128, 128], F32)
            make_identity(nc, ident[:])

            # long-lived tensors (~90KB/partition)
            qTh = [pp.tile([128, S], F32R, name=f"qTh{h}") for h in range(HPC)]
            kTh = [pp.tile([128, S], F32R, name=f"kTh{h}") for h in range(HPC)]
            v_t = [pp.tile([128, JC], F32R, name=f"v{st}") for st in range(ST)]
            dnT = [pp.tile([128, D], F32R, name=f"dnT{p}") for p in range(2)]
            maskbT = pp.tile([128, 16], F32)
            rowrec = [pp.tile([128, 16], F32, name=f"rr{h}") for h in range(HPC)]
            qb_t = pp.tile([128, 2], F32)
            kb_t = pp.tile([128, 2], F32)

            # ============ early section: masks, weights, xT, projections
            with tc.tile_pool(name="early", bufs=1) as ep:
                xT = [ep.tile([128, S], F32R, name=f"xT{c}") for c in range(IC)]
                wqT = [ep.tile([128, JC], F32R, name=f"wqT{c}") for c in range(IC)]
                wkT = [ep.tile([128, JC], F32R, name=f"wkT{c}") for c in range(IC)]
                wvT = [ep.tile([128, JC], F32R, name=f"wvT{c}") for c in range(IC)]

                # ---- masks / biases ---------------------------------
                mrow_r = ep.tile([1, S], F32R)
                with tc.tile_pool(name="msktmp", bufs=1) as mp:
                    mrow_i = mp.tile([1, S], I32)
                    nc.sync.dma_start(mrow_i[:], maskb[:].unsqueeze(0))
                    mrow_f = mp.tile([1, S], F32)
                    nc.vector.tensor_copy(mrow_f[:], mrow_i[:])
                    nc.vector.tensor_scalar_mul(mrow_r[:], mrow_f[:], NEG)

                    mcol_i = mp.tile([16, 128], I32)
                    nc.sync.dma_start(
                        mcol_i[:], maskb[:].rearrange("(a b) -> a b", b=128))
                    mcol_f = mp.tile([16, 128], F32)
                    nc.vector.tensor_copy(mcol_f[:], mcol_i[:])
                    with tc.tile_pool(name="ps0", bufs=1, space="PSUM") as ps0:
                        mcol_t = ps0.tile([128, 16], F32)
                        nc.tensor.transpose(
                            mcol_t[:], mcol_f[:], ident[0:16, 0:16])
                        nc.vector.tensor_scalar_mul(
                            maskbT[:], mcol_t[:], NEG * SCALE)

                for t in range(2):
                    nc.sync.dma_start(
                        qb_t[:, t:t + 1],
                        wqb[t * 128:(t + 1) * 128].rearrange("(p o) -> p o", o=1))
                    nc.sync.dma_start(
                        kb_t[:, t:t + 1],
                        wkb[t * 128:(t + 1) * 128].rearrange("(p o) -> p o", o=1))

                # ---- weight transposes ------------------------------
                with (
                    tc.tile_pool(name="wload", bufs=2) as wl,
                    tc.tile_pool(name="psw", bufs=4, space="PSUM") as psw,
                ):
                    for (w_dram, wT) in ((wq, wqT), (wk, wkT), (wv, wvT)):
                        for jt in range(2):
                            wt = wl.tile([128, D], F32, tag="wnat")
                            nc.sync.dma_start(
                                wt[:], w_dram[jt * 128:(jt + 1) * 128, :])
                            for c in range(IC):
                                tp = psw.tile([128, 128], F32, tag="wtp")
                                nc.tensor.transpose(
                                    tp[:], wt[:, c * 128:(c + 1) * 128],
                                    ident[:])
                                nc.vector.tensor_copy(
                                    wT[c][:, jt * 128:(jt + 1) * 128], tp[:])
                    for mt in range(IC):
                        dt_ = wl.tile([128, JC], F32, tag="dnat")
                        nc.sync.dma_start(
                            dt_[:], dns[mt * 128:(mt + 1) * 128, :])
                        for jb in range(2):
                            tp = psw.tile([128, 128], F32, tag="wtp")
                            nc.tensor.transpose(
                                tp[:], dt_[:, jb * 128:(jb + 1) * 128],
                                ident[:])
                            nc.vector.tensor_copy(
                                dnT[jb][:, mt * 128:(mt + 1) * 128], tp[:])

                # ---- phase 1: xT (streamed, st-outer) ---------------
                with (
                    tc.tile_pool(name="xload", bufs=3) as xl,
                    tc.tile_pool(name="psx", bufs=4, space="PSUM") as psx,
                ):
                    for st in range(ST):
                        xt = xl.tile([128, D], F32, tag="xnat")
                        nc.sync.dma_start(xt[:], x[st * 128:(st + 1) * 128, :])
                        for c in range(IC):
                            tp = psx.tile([128, 128], F32, tag="xtp")
                            nc.tensor.transpose(
                                tp[:], xt[:, c * 128:(c + 1) * 128], ident[:])
                            nc.vector.tensor_copy(
                                xT[c][:, st * 128:(st + 1) * 128], tp[:])

                # ---- phase 2: projections ---------------------------
                for h in range(HPC):
                    nc.vector.memset(qTh[h][:].bitcast(F32), 0.0)
                    nc.vector.memset(kTh[h][:].bitcast(F32), 0.0)
                    nc.vector.memset(qTh[h][64:65, :].bitcast(F32), 1.0)
                    nc.gpsimd.tensor_copy(kTh[h][64:65, :], mrow_r[:])

                with (
                    tc.tile_pool(name="projtmp", bufs=3) as pt,
                    tc.tile_pool(name="psp", bufs=2, space="PSUM") as psp,
                ):
                    for (wT, bias, dsth) in ((wqT, qb_t, qTh), (wkT, kb_t, kTh)):
                        for jt in range(2):
                            for s4 in range(4):
                                pj = psp.tile([128, 512], F32, tag="pj")
                                for c in range(IC):
                                    nc.tensor.matmul(
                                        pj[:],
                                        wT[c][:, jt * 128:(jt + 1) * 128],
                                        xT[c][:, s4 * 512:(s4 + 1) * 512],
                                        start=(c == 0), stop=(c == IC - 1))
                                tmp = pt.tile([128, 512], F32R, tag="pe")
                                nc.vector.tensor_scalar_add(
                                    tmp[:], pj[:], bias[:, jt:jt + 1])
                                sl = slice(s4 * 512, (s4 + 1) * 512)
                                nc.gpsimd.tensor_copy(
                                    dsth[2 * jt][0:64, sl], tmp[0:64, :])
                                nc.gpsimd.tensor_copy(
                                    dsth[2 * jt + 1][0:64, sl], tmp[64:128, :])
                    for st in range(ST):
                        pv = psp.tile([128, JC], F32, tag="pv")
                        for c in range(IC):
                            nc.tensor.matmul(
                                pv[:], xT[c][:, st * 128:(st + 1) * 128],
                                wvT[c][:], start=(c == 0), stop=(c == IC - 1))
                        nc.vector.tensor_copy(v_t[st][:], pv[:])
            # ============ end early section (frees xT + weight tiles)

            with tc.tile_pool(name="late", bufs=1) as lp:
                rep = [lp.tile([64, S], F32, name=f"rep{h}")
                       for h in range(HPC)]
                ctxT = [lp.tile([128, S], F32R, name=f"ctxT{p}")
                        for p in range(2)]

                # ---- phase A: natural softmax + attn out ------------
                with (
                    tc.tile_pool(name="attA", bufs=3) as pa,
                    tc.tile_pool(name="psA", bufs=2, space="PSUM") as psA,
                ):
                    for h in range(HPC):
                        for qt in range(ST):
                            pl = psA.tile([128, S], F32, tag="pl")
                            for kt in range(4):
                                nc.tensor.matmul(
                                    pl[:, kt * 512:(kt + 1) * 512],
                                    qTh[h][:, qt * 128:(qt + 1) * 128],
                                    kTh[h][:, kt * 512:(kt + 1) * 512],
                                    start=True, stop=True)
                            un = pa.tile([128, S], F32, tag="un")
                            rs = pa.tile([128, 1], F32, tag="rs")
                            nc.scalar.activation(
                                un[:], pl[:],
                                mybir.ActivationFunctionType.Exp,
                                scale=SCALE, accum_out=rs[:])
                            nc.vector.reciprocal(rowrec[h][:, qt:qt + 1], rs[:])
                            ao = pa.tile([128, S], F32, tag="ao")
                            nc.vector.tensor_scalar_mul(
                                ao[:], un[:], rowrec[h][:, qt:qt + 1])
                            nc.sync.dma_start(
                                attn_p[h, qt * 128:(qt + 1) * 128, :], ao[:])

                # ---- recip rows (transposed + replicated) -----------
                with (
                    tc.tile_pool(name="rtmp", bufs=2) as rt,
                    tc.tile_pool(name="psR", bufs=2, space="PSUM") as psR,
                ):
                    for h in range(HPC):
                        tp = psR.tile([16, 128], F32, tag="rtp")
                        nc.tensor.transpose(tp[:], rowrec[h][:], ident[:])
                        sb16 = rt.tile([16, 128], F32, tag="r16")
                        nc.vector.tensor_copy(sb16[:], tp[:])
                        scr = dpool.tile([16, 128], F32, name=f"scr{h}",
                                         tag=f"scr{h}")
                        nc.sync.dma_start(scr[:], sb16[:])
                        flat = rt.tile([1, S], F32, tag="rflat")
                        nc.sync.dma_start(
                            flat[:],
                            scr[:].rearrange("a b -> (a b)").unsqueeze(0))
                        nc.gpsimd.partition_broadcast(rep[h][:], flat[:])

                # ---- phase B: transposed exp + AV -------------------
                with (
                    tc.tile_pool(name="attB", bufs=4) as pb,
                    tc.tile_pool(name="psQ", bufs=2, space="PSUM") as psQ,
                    tc.tile_pool(name="psV", bufs=1, space="PSUM") as psV,
                ):
                    for h in range(HPC):
                        pav = [psV.tile([64, 512], F32, tag=f"pav{qg}",
                                        name=f"pav{qg}")
                               for qg in range(4)]
                        for kc in range(ST):
                            for qh in range(2):
                                pq = psQ.tile([128, 1024], F32, tag="pq")
                                for qi in range(2):
                                    nc.tensor.matmul(
                                        pq[:, qi * 512:(qi + 1) * 512],
                                        kTh[h][0:64, kc * 128:(kc + 1) * 128],
                                        qTh[h][0:64,
                                               qh * 1024 + qi * 512:
                                               qh * 1024 + (qi + 1) * 512],
                                        start=True, stop=True)
                                eT = pb.tile([128, 1024], F32R, tag="eT")
                                nc.scalar.activation(
                                    eT[:], pq[:],
                                    mybir.ActivationFunctionType.Exp,
                                    scale=SCALE, bias=maskbT[:, kc:kc + 1])
                                for qi in range(2):
                                    qg = qh * 2 + qi
                                    nc.tensor.matmul(
                                        pav[qg][:],
                                        v_t[kc][:, h * 64:(h + 1) * 64],
                                        eT[:, qi * 512:(qi + 1) * 512],
                                        start=(kc == 0), stop=(kc == ST - 1))
                        p, lo = h // 2, h % 2
                        for qg in range(4):
                            sl = slice(qg * 512, (qg + 1) * 512)
                            if lo == 0:
                                nc.vector.tensor_tensor(
                                    ctxT[p][0:64, sl], pav[qg][:],
                                    rep[h][:, sl], mybir.AluOpType.mult)
                            else:
                                stg = pb.tile([64, 512], F32R, tag="stg")
                                nc.vector.tensor_tensor(
                                    stg[:], pav[qg][:],
                                    rep[h][:, sl], mybir.AluOpType.mult)
                                nc.sync.dma_start(ctxT[p][64:128, sl], stg[:])

                # ---- phase C: output projection ---------------------
                with (
                    tc.tile_pool(name="outC", bufs=3) as pc,
                    tc.tile_pool(name="psC", bufs=4, space="PSUM") as psC,
                ):
                    for st in range(ST):
                        ob = pc.tile([128, D], F32, tag="ob")
                        for m in range(2):
                            po = psC.tile([128, 512], F32, tag="po")
                            for p in range(2):
                                nc.tensor.matmul(
                                    po[:],
                                    ctxT[p][:, st * 128:(st + 1) * 128],
                                    dnT[p][:, m * 512:(m + 1) * 512],
                                    start=(p == 0), stop=(p == 1))
                            nc.vector.tensor_copy(
                                ob[:, m * 512:(m + 1) * 512], po[:])
                        nc.sync.dma_start(
                            out_p[st * 128:(st + 1) * 128, :], ob[:])

    nc.finalize()
    return nc


def _get_nc():
    global _NC_CACHE
    if _NC_CACHE is None:
        _NC_CACHE = build_nc()
    return _NC_CACHE


def kernel(x, mask, wq_w, wq_b, wk_w, wk_b, wv_w, wv_b, dense_w, dense_b,
           _trace=False):
    x = np.asarray(x, dtype=np.float32)
    mask = np.asarray(mask, dtype=np.int32)
    wq_w = np.asarray(wq_w, dtype=np.float32)
    wq_b = np.asarray(wq_b, dtype=np.float32)
    wk_w = np.asarray(wk_w, dtype=np.float32)
    wk_b = np.asarray(wk_b, dtype=np.float32)
    wv_w = np.asarray(wv_w, dtype=np.float32)
    wv_b = np.asarray(wv_b, dtype=np.float32)
    dense_w = np.asarray(dense_w, dtype=np.float32)
    dense_b = np.asarray(dense_b, dtype=np.float32)

    nc = _get_nc()
    in_maps = []
    for c in range(NCORES):
        b, g = c // HPC, c % HPC
        js = slice(g * JC, (g + 1) * JC)
        in_maps.append({
            "x": np.ascontiguousarray(x[b]),
            "maskb": np.ascontiguousarray(mask[b]),
            "wq": np.ascontiguousarray(wq_w[js]),
            "wk": np.ascontiguousarray(wk_w[js]),
            "wv": np.ascontiguousarray(wv_w[js]),
            "wqb": np.ascontiguousarray(wq_b[js]),
            "wkb": np.ascontiguousarray(wk_b[js]),
            "dns": np.ascontiguousarray(dense_w[:, js]),
        })

    res = run_bass_kernel_spmd(
        nc, in_maps, core_ids=list(range(NCORES)), trace=_trace)

    attn = np.empty((B, H, S, S), dtype=np.float32)
    out = np.empty((B, S, D), dtype=np.float32)
    # constant contribution of the V bias through the dense layer
    # (attention rows sum to 1 -> ctx += wv_b exactly)
    const_row = wv_b @ dense_w.T + dense_b
    for b in range(B):
        acc = np.zeros((S, D), dtype=np.float32)
        for g in range(HPC):
            c = b * HPC + g
            attn[b, g * HPC:(g + 1) * HPC] = res.results[c]["attn_p"]
            acc += res.results[c]["out_p"]
        out[b] = acc + const_row[None, :]

    if _trace:
        return (out, attn), res
    return out, attn


# revision 8
# speedup vs baseline: 1.0577x; 1.0577x over previous
"""Multi-head attention Trainium2 kernel (B=2, S=2048, D=1024, H=16).

Sharding: 8 cores, each handles (batch b = core//4, head group g = core%4,
heads 4g..4g+3). Returns full (out, attn) like the reference.

Per-core plan (matmuls in fp32r = 1 cyc/row at N>=256):
  phase 0: masks/biases, identity, weight transposes (PE)
  phase 1: transpose x_b -> xT [1024, 2048] via PE
  phase 2: projections q'T/k'T (per-head padded [128,2048] tiles, ones/mask
           row at partition 64), v natural [2048, 256]
  phase A (per head): natural logits = q'T.T @ k'T (K=128 incl. mask row),
           exp on ACT with accum_out row sums, normalize on DVE, DMA attn out
  phase B (per head): transposed logits kT.T@qT per k-chunk, exp with
           per-partition mask bias, AV matmuls accumulate ctxT in PSUM,
           scale by broadcast recip rows
  phase C: out_partial = ctxT.T @ denseT, DMA out
Host: sums the 4 partial outs per batch (+ constant bias term), stacks attn.
"""
import os
import sys

if "/opt/trn_rl_repo" not in sys.path:
    sys.path.insert(0, "/opt/trn_rl_repo")

import numpy as np

import concourse.bass as bass
import concourse.tile as tile
from concourse import mybir, bacc
from concourse.bass_utils import run_bass_kernel_spmd
from concourse.masks import make_identity

F32 = mybir.dt.float32
F32R = mybir.dt.float32r
BF16 = mybir.dt.bfloat16
I32 = mybir.dt.int32

B, S, D, H = 2, 2048, 1024, 16
DH = 64            # head dim
HPC = 4            # heads per core
JC = HPC * DH      # 256 j-dims per core
NCORES = 8
NEG = -1.0e9       # additive mask constant (pre-scale)
SCALE = 0.125      # 1/sqrt(64)

_NC_CACHE = None


def build_nc():
    nc = bacc.Bacc()

    x = nc.dram_tensor("x", [S, D], F32, kind="ExternalInput")
    maskb = nc.dram_tensor("maskb", [S], I32, kind="ExternalInput")
    wq = nc.dram_tensor("wq", [JC, D], F32, kind="ExternalInput")
    wk = nc.dram_tensor("wk", [JC, D], F32, kind="ExternalInput")
    wv = nc.dram_tensor("wv", [JC, D], F32, kind="ExternalInput")
    wqb = nc.dram_tensor("wqb", [JC], F32, kind="ExternalInput")
    wkb = nc.dram_tensor("wkb", [JC], F32, kind="ExternalInput")
    dns = nc.dram_tensor("dns", [D, JC], F32, kind="ExternalInput")
    attn_p = nc.dram_tensor("attn_p", [HPC, S, S], F32, kind="ExternalOutput")
    out_p = nc.dram_tensor("out_p", [S, D], F32, kind="ExternalOutput")

    ST = S // 128   # 16 s-tiles
    IC = D // 128   # 8 input-feature chunks

    with tile.TileContext(nc) as tc:
        with (
            tc.tile_pool(name="life", bufs=1) as pp,
            tc.tile_pool(name="dram", bufs=1, space="DRAM") as dpool,
        ):
            ident = pp.tile([128, 128], F32)
            make_identity(nc, ident[:])

            # long-lived tensors (~90KB/partition)
            qTh = [pp.tile([128, S], F32R, name=f"qTh{h}") for h in range(HPC)]
            kTh = [pp.tile([128, S], F32R, name=f"kTh{h}") for h in range(HPC)]
            v_t = [pp.tile([128, JC], BF16, name=f"v{st}") for st in range(ST)]
            dnT = [pp.tile([128, D], BF16, name=f"dnT{p}") for p in range(2)]
            maskbT = pp.tile([128, 16], F32)
            rowrec = [pp.tile([128, 16], F32, name=f"rr{h}") for h in range(HPC)]
            qb_t = pp.tile([128, 2], F32)
            kb_t = pp.tile([128, 2], F32)

            # ============ early section: masks, weights, xT, projections
            with tc.tile_pool(name="early", bufs=1) as ep:
                xT = [ep.tile([128, S], F32R, name=f"xT{c}") for c in range(IC)]
                wqT = [ep.tile([128, JC], F32R, name=f"wqT{c}") for c in range(IC)]
                wkT = [ep.tile([128, JC], F32R, name=f"wkT{c}") for c in range(IC)]
                wvT = [ep.tile([128, JC], F32R, name=f"wvT{c}") for c in range(IC)]

                # ---- masks / biases ---------------------------------
                mrow_r = ep.tile([1, S], F32R)
                with tc.tile_pool(name="msktmp", bufs=1) as mp:
                    mrow_i = mp.tile([1, S], I32)
                    nc.sync.dma_start(mrow_i[:], maskb[:].unsqueeze(0))
                    mrow_f = mp.tile([1, S], F32)
                    nc.vector.tensor_copy(mrow_f[:], mrow_i[:])
                    nc.vector.tensor_scalar_mul(mrow_r[:], mrow_f[:], NEG)

                    mcol_i = mp.tile([16, 128], I32)
                    nc.sync.dma_start(
                        mcol_i[:], maskb[:].rearrange("(a b) -> a b", b=128))
                    mcol_f = mp.tile([16, 128], F32)
                    nc.vector.tensor_copy(mcol_f[:], mcol_i[:])
                    with tc.tile_pool(name="ps0", bufs=1, space="PSUM") as ps0:
                        mcol_t = ps0.tile([128, 16], F32)
                        nc.tensor.transpose(
                            mcol_t[:], mcol_f[:], ident[0:16, 0:16])
                        nc.vector.tensor_scalar_mul(
                            maskbT[:], mcol_t[:], NEG * SCALE)

                for t in range(2):
                    nc.sync.dma_start(
                        qb_t[:, t:t + 1],
                        wqb[t * 128:(t + 1) * 128].rearrange("(p o) -> p o", o=1))
                    nc.sync.dma_start(
                        kb_t[:, t:t + 1],
                        wkb[t * 128:(t + 1) * 128].rearrange("(p o) -> p o", o=1))

                # ---- weight transposes ------------------------------
                with (
                    tc.tile_pool(name="wload", bufs=2) as wl,
                    tc.tile_pool(name="psw", bufs=4, space="PSUM") as psw,
                ):
                    for (w_dram, wT) in ((wq, wqT), (wk, wkT), (wv, wvT)):
                        for jt in range(2):
                            wt = wl.tile([128, D], F32, tag="wnat")
                            nc.sync.dma_start(
                                wt[:], w_dram[jt * 128:(jt + 1) * 128, :])
                            for c in range(IC):
                                tp = psw.tile([128, 128], F32, tag="wtp")
                                nc.tensor.transpose(
                                    tp[:], wt[:, c * 128:(c + 1) * 128],
                                    ident[:])
                                nc.vector.tensor_copy(
                                    wT[c][:, jt * 128:(jt + 1) * 128], tp[:])
                    for mt in range(IC):
                        dt_ = wl.tile([128, JC], F32, tag="dnat")
                        nc.sync.dma_start(
                            dt_[:], dns[mt * 128:(mt + 1) * 128, :])
                        for jb in range(2):
                            tp = psw.tile([128, 128], F32, tag="wtp")
                            nc.tensor.transpose(
                                tp[:], dt_[:, jb * 128:(jb + 1) * 128],
                                ident[:])
                            nc.vector.tensor_copy(
                                dnT[jb][:, mt * 128:(mt + 1) * 128], tp[:])

                # ---- phase 1: xT (streamed, st-outer) ---------------
                with (
                    tc.tile_pool(name="xload", bufs=3) as xl,
                    tc.tile_pool(name="psx", bufs=4, space="PSUM") as psx,
                ):
                    for st in range(ST):
                        xt = xl.tile([128, D], F32, tag="xnat")
                        nc.sync.dma_start(xt[:], x[st * 128:(st + 1) * 128, :])
                        for c in range(IC):
                            tp = psx.tile([128, 128], F32, tag="xtp")
                            nc.tensor.transpose(
                                tp[:], xt[:, c * 128:(c + 1) * 128], ident[:])
                            nc.vector.tensor_copy(
                                xT[c][:, st * 128:(st + 1) * 128], tp[:])

                # ---- phase 2: projections ---------------------------
                for h in range(HPC):
                    nc.vector.memset(qTh[h][64:128, :].bitcast(F32), 0.0)
                    nc.vector.memset(kTh[h][64:128, :].bitcast(F32), 0.0)
                    nc.vector.memset(qTh[h][64:65, :].bitcast(F32), 1.0)
                    nc.sync.dma_start(kTh[h][64:65, :], mrow_r[:])

                with (
                    tc.tile_pool(name="projtmp", bufs=3) as pt,
                    tc.tile_pool(name="psp", bufs=2, space="PSUM") as psp,
                ):
                    for (wT, bias, dsth) in ((wqT, qb_t, qTh), (wkT, kb_t, kTh)):
                        for jt in range(2):
                            for s4 in range(4):
                                pj = psp.tile([128, 512], F32, tag="pj")
                                for c in range(IC):
                                    nc.tensor.matmul(
                                        pj[:],
                                        wT[c][:, jt * 128:(jt + 1) * 128],
                                        xT[c][:, s4 * 512:(s4 + 1) * 512],
                                        start=(c == 0), stop=(c == IC - 1))
                                tmp = pt.tile([128, 512], F32R, tag="pe")
                                nc.vector.tensor_scalar_add(
                                    tmp[:], pj[:], bias[:, jt:jt + 1])
                                sl = slice(s4 * 512, (s4 + 1) * 512)
                                nc.sync.dma_start(
                                    dsth[2 * jt][0:64, sl], tmp[0:64, :])
                                nc.sync.dma_start(
                                    dsth[2 * jt + 1][0:64, sl], tmp[64:128, :])
                    for st in range(ST):
                        pv = psp.tile([128, JC], F32, tag="pv")
                        for c in range(IC):
                            nc.tensor.matmul(
                                pv[:], xT[c][:, st * 128:(st + 1) * 128],
                                wvT[c][:], start=(c == 0), stop=(c == IC - 1))
                        nc.vector.tensor_copy(v_t[st][:], pv[:])
            # ============ end early section (frees xT + weight tiles)

            with tc.tile_pool(name="late", bufs=1) as lp:
                rep = [lp.tile([64, S], F32, name=f"rep{h}")
                       for h in range(HPC)]
                ctxT = [lp.tile([128, S], BF16, name=f"ctxT{p}")
                        for p in range(2)]
                qbTh = [lp.tile([64, S], BF16, name=f"qbTh{h}")
                        for h in range(HPC)]
                kbTh = [lp.tile([64, S], BF16, name=f"kbTh{h}")
                        for h in range(HPC)]
                for h in range(HPC):
                    nc.vector.tensor_copy(qbTh[h][:], qTh[h][0:64, :])
                    nc.vector.tensor_copy(kbTh[h][:], kTh[h][0:64, :])

                # ---- phase A: natural softmax + attn out ------------
                with (
                    tc.tile_pool(name="attA", bufs=3) as pa,
                    tc.tile_pool(name="psA", bufs=2, space="PSUM") as psA,
                ):
                    for h in range(HPC):
                        for qt in range(ST):
                            pl = psA.tile([128, S], F32, tag="pl")
                            for kt in range(4):
                                nc.tensor.matmul(
                                    pl[:, kt * 512:(kt + 1) * 512],
                                    qTh[h][:, qt * 128:(qt + 1) * 128],
                                    kTh[h][:, kt * 512:(kt + 1) * 512],
                                    start=True, stop=True)
                            un = pa.tile([128, S], F32, tag="un")
                            rs = pa.tile([128, 1], F32, tag="rs")
                            nc.scalar.activation(
                                un[:], pl[:],
                                mybir.ActivationFunctionType.Exp,
                                scale=SCALE, accum_out=rs[:])
                            nc.vector.reciprocal(rowrec[h][:, qt:qt + 1], rs[:])
                            ao = pa.tile([128, S], F32, tag="ao")
                            nc.vector.tensor_scalar_mul(
                                ao[:], un[:], rowrec[h][:, qt:qt + 1])
                            nc.sync.dma_start(
                                attn_p[h, qt * 128:(qt + 1) * 128, :], ao[:])

                # ---- recip rows (transposed + replicated) -----------
                with (
                    tc.tile_pool(name="rtmp", bufs=2) as rt,
                    tc.tile_pool(name="psR", bufs=2, space="PSUM") as psR,
                ):
                    for h in range(HPC):
                        tp = psR.tile([16, 128], F32, tag="rtp")
                        nc.tensor.transpose(tp[:], rowrec[h][:], ident[:])
                        sb16 = rt.tile([16, 128], F32, tag="r16")
                        nc.vector.tensor_copy(sb16[:], tp[:])
                        scr = dpool.tile([16, 128], F32, name=f"scr{h}",
                                         tag=f"scr{h}")
                        nc.sync.dma_start(scr[:], sb16[:])
                        flat = rt.tile([1, S], F32, tag="rflat")
                        nc.sync.dma_start(
                            flat[:],
                            scr[:].rearrange("a b -> (a b)").unsqueeze(0))
                        nc.gpsimd.partition_broadcast(rep[h][:], flat[:])

                # ---- phase B: transposed exp + AV -------------------
                with (
                    tc.tile_pool(name="attB", bufs=4) as pb,
                    tc.tile_pool(name="psQ", bufs=2, space="PSUM") as psQ,
                    tc.tile_pool(name="psV", bufs=1, space="PSUM") as psV,
                ):
                    for h in range(HPC):
                        pav = [psV.tile([64, 512], F32, tag=f"pav{qg}",
                                        name=f"pav{qg}")
                               for qg in range(4)]
                        for kc in range(ST):
                            for qh in range(2):
                                pq = psQ.tile([128, 1024], F32, tag="pq")
                                for qi in range(2):
                                    nc.tensor.matmul(
                                        pq[:, qi * 512:(qi + 1) * 512],
                                        kbTh[h][:, kc * 128:(kc + 1) * 128],
                                        qbTh[h][:,
                                                qh * 1024 + qi * 512:
                                                qh * 1024 + (qi + 1) * 512],
                                        start=True, stop=True)
                                eT = pb.tile([128, 1024], BF16, tag="eT")
                                nc.scalar.activation(
                                    eT[:], pq[:],
                                    mybir.ActivationFunctionType.Exp,
                                    scale=SCALE, bias=maskbT[:, kc:kc + 1])
                                for qi in range(2):
                                    qg = qh * 2 + qi
                                    nc.tensor.matmul(
                                        pav[qg][:],
                                        v_t[kc][:, h * 64:(h + 1) * 64],
                                        eT[:, qi * 512:(qi + 1) * 512],
                                        start=(kc == 0), stop=(kc == ST - 1))
                        p, lo = h // 2, h % 2
                        for qg in range(4):
                            sl = slice(qg * 512, (qg + 1) * 512)
                            if lo == 0:
                                nc.vector.tensor_tensor(
                                    ctxT[p][0:64, sl], pav[qg][:],
                                    rep[h][:, sl], mybir.AluOpType.mult)
                            else:
                                stg = pb.tile([64, 512], BF16, tag="stg")
                                nc.vector.tensor_tensor(
                                    stg[:], pav[qg][:],
                                    rep[h][:, sl], mybir.AluOpType.mult)
                                nc.sync.dma_start(ctxT[p][64:128, sl], stg[:])

                # ---- phase C: output projection ---------------------
                with (
                    tc.tile_pool(name="outC", bufs=3) as pc,
                    tc.tile_pool(name="psC", bufs=4, space="PSUM") as psC,
                ):
                    for st in range(ST):
                        ob = pc.tile([128, D], F32, tag="ob")
                        for m in range(2):
                            po = psC.tile([128, 512], F32, tag="po")
                            for p in range(2):
                                nc.tensor.matmul(
                                    po[:],
                                    ctxT[p][:, st * 128:(st + 1) * 128],
                                    dnT[p][:, m * 512:(m + 1) * 512],
                                    start=(p == 0), stop=(p == 1))
                            nc.vector.tensor_copy(
                                ob[:, m * 512:(m + 1) * 512], po[:])
                        nc.sync.dma_start(
                            out_p[st * 128:(st + 1) * 128, :], ob[:])

    nc.finalize()
    return nc


def _get_nc():
    global _NC_CACHE
    if _NC_CACHE is None:
        _NC_CACHE = build_nc()
    return _NC_CACHE


def kernel(x, mask, wq_w, wq_b, wk_w, wk_b, wv_w, wv_b, dense_w, dense_b,
           _trace=False):
    x = np.asarray(x, dtype=np.float32)
    mask = np.asarray(mask, dtype=np.int32)
    wq_w = np.asarray(wq_w, dtype=np.float32)
    wq_b = np.asarray(wq_b, dtype=np.float32)
    wk_w = np.asarray(wk_w, dtype=np.float32)
    wk_b = np.asarray(wk_b, dtype=np.float32)
    wv_w = np.asarray(wv_w, dtype=np.float32)
    wv_b = np.asarray(wv_b, dtype=np.float32)
    dense_w = np.asarray(dense_w, dtype=np.float32)
    dense_b = np.asarray(dense_b, dtype=np.float32)

    nc = _get_nc()
    in_maps = []
    for c in range(NCORES):
        b, g = c // HPC, c % HPC
        js = slice(g * JC, (g + 1) * JC)
        in_maps.append({
            "x": np.ascontiguousarray(x[b]),
            "maskb": np.ascontiguousarray(mask[b]),
            "wq": np.ascontiguousarray(wq_w[js]),
            "wk": np.ascontiguousarray(wk_w[js]),
            "wv": np.ascontiguousarray(wv_w[js]),
            "wqb": np.ascontiguousarray(wq_b[js]),
            "wkb": np.ascontiguousarray(wk_b[js]),
            "dns": np.ascontiguousarray(dense_w[:, js]),
        })

    res = run_bass_kernel_spmd(
        nc, in_maps, core_ids=list(range(NCORES)), trace=_trace)

    attn = np.empty((B, H, S, S), dtype=np.float32)
    out = np.empty((B, S, D), dtype=np.float32)
    # constant contribution of the V bias through the dense layer
    # (attention rows sum to 1 -> ctx += wv_b exactly)
    const_row = wv_b @ dense_w.T + dense_b
    for b in range(B):
        acc = np.zeros((S, D), dtype=np.float32)
        for g in range(HPC):
            c = b * HPC + g
            attn[b, g * HPC:(g + 1) * HPC] = res.results[c]["attn_p"]
            acc += res.results[c]["out_p"]
        out[b] = acc + const_row[None, :]

    if _trace:
        return (out, attn), res
    return out, attn
